# revision 1
# baseline (speedup 1.0000x reference)
"""Adaptive-softmax NLL loss on 8 TRN2 NeuronCores.

Strategy: tensor-parallel over the vocab dimension. Each core computes the
exp-sums of its vocab slice of head / tail1 / tail2 logits for all 4096
tokens, plus (token-sharded) the gathered target-logit dot products. One
small AllReduce combines per-token sum-exp partials; every core then
finishes the scalar NLL identically.

NLL = sum_n log(S_head_n) + sum_{n in c1} log(S_t1_n) + sum_{n in c2} log(S_t2_n)
      - sum_n x_n . W_ext[cidx_n] - sum_{c1} h1_n . W1[t_n-C0] - sum_{c2} h2_n . W2[t_n-C1]

where S_* are softmax denominators (no max-subtraction needed: logits are
O(1) by construction), cidx_n = target_n if < C0 else C0/C0+1 (cluster
prior column), and h1/h2 are the low-rank projections computed on device.
Host-side work is limited to index gathers / layout transforms of inputs.
"""

import os
import sys

for _p in ("/opt/trn_rl_repo",):
    if _p not in sys.path:
        sys.path.insert(0, _p)

import numpy as np

import concourse.bacc as bacc
import concourse.bass as bass
import concourse.mybir as mybir
import concourse.tile as tile
from concourse.bass_utils import run_bass_kernel_spmd

dt = mybir.dt
AF = mybir.ActivationFunctionType
ALU = mybir.AluOpType

NCORES = 8
N, D = 4096, 1024
C0, C1, C2 = 20000, 40000, 50257
VH = C0 + 2          # head logits incl 2 cluster columns
R1, R2 = 256, 64
VHC = 2560           # head vocab rows per core (8*2560 = 20480, pad 478)
V1C = 2560           # tail1 rows per core   (8*2560 = 20480, pad 480)
V2C = 1536           # tail2 rows per core   (8*1536 = 12288, pad 2031)
PAD_H = NCORES * VHC - VH
PAD_1 = NCORES * V1C - (C1 - C0)
PAD_2 = NCORES * V2C - (C2 - C1)
NT = N // 128        # 32 token tiles
NSH = N // NCORES    # 512 tokens per core for the sharded head dot

F32, F32R = dt.float32, dt.float32r

LAST_EXEC_NS = None


def _build(phases=4):
    nc = bacc.Bacc("TRN2", target_bir_lowering=False, debug=False,
                   num_devices=NCORES)

    xT = nc.declare_dram_parameter("xT", [D, N], F32, isOutput=False)
    whT = nc.declare_dram_parameter("whT", [D, VHC], F32, isOutput=False)
    w1T = nc.declare_dram_parameter("w1T", [R1, V1C], F32, isOutput=False)
    w2T = nc.declare_dram_parameter("w2T", [R2, V2C], F32, isOutput=False)
    p1T = nc.declare_dram_parameter("p1T", [D, R1], F32, isOutput=False)
    p2T = nc.declare_dram_parameter("p2T", [D, R2], F32, isOutput=False)
    xTc = nc.declare_dram_parameter("xTc", [D, NSH], F32, isOutput=False)
    wselT = nc.declare_dram_parameter("wselT", [D, NSH], F32, isOutput=False)
    w1selT = nc.declare_dram_parameter("w1selT", [R1, N], F32, isOutput=False)
    w2selT = nc.declare_dram_parameter("w2selT", [R2, N], F32, isOutput=False)
    m1_in = nc.declare_dram_parameter("m1", [128, NT], F32, isOutput=False)
    m2_in = nc.declare_dram_parameter("m2", [128, NT], F32, isOutput=False)
    out_ext = nc.declare_dram_parameter("out", [1, 1], F32, isOutput=True)

    KD = D // 128  # 8 k-tiles over the D contraction

    with tile.TileContext(nc) as tc:
        with (
            tc.tile_pool(name="res", bufs=1) as res,       # resident tensors
            tc.tile_pool(name="dram", bufs=1, space="DRAM") as dram,
        ):
            # ---- resident loads -------------------------------------------------
            whT_sb = res.tile([128, KD * VHC], F32R)   # [p, (k v)]
            nc.sync.dma_start(
                out=whT_sb[:].rearrange("p (k v) -> p k v", k=KD),
                in_=whT.ap().bitcast(F32R).rearrange("(k p) v -> p k v", p=128))
            w1T_sb = res.tile([128, 2 * V1C], F32R)
            nc.sync.dma_start(
                out=w1T_sb[:].rearrange("p (k v) -> p k v", k=2),
                in_=w1T.ap().bitcast(F32R).rearrange("(k p) v -> p k v", p=128))
            w2T_sb = res.tile([64, V2C], F32R)
            nc.sync.dma_start(out=w2T_sb[:], in_=w2T.ap().bitcast(F32R))
            p1T_sb = res.tile([128, KD * R1], F32R)
            nc.sync.dma_start(
                out=p1T_sb[:].rearrange("p (k r) -> p k r", k=KD),
                in_=p1T.ap().bitcast(F32R).rearrange("(k p) r -> p k r", p=128))
            p2T_sb = res.tile([128, KD * R2], F32R)
            nc.sync.dma_start(
                out=p2T_sb[:].rearrange("p (k r) -> p k r", k=KD),
                in_=p2T.ap().bitcast(F32R).rearrange("(k p) r -> p k r", p=128))
            m1_sb = res.tile([128, NT], F32)
            nc.sync.dma_start(out=m1_sb[:], in_=m1_in.ap())
            m2_sb = res.tile([128, NT], F32)
            nc.sync.dma_start(out=m2_sb[:], in_=m2_in.ap())

            h1T_sb = [res.tile([128, N], F32R, tag=f"h1T{r}", name=f"h1T{r}")
                      for r in range(2)]
            h2T_sb = res.tile([64, N], F32R)

            sh_slots = res.tile([128, NT], F32)
            s1_slots = res.tile([128, NT], F32)
            s2_slots = res.tile([128, NT], F32)
            dsh_slots = res.tile([128, KD], F32)   # sharded head dot partials
            dgl_slots = res.tile([128, 8], F32)    # unsharded t1/t2 dot partials
            nc.vector.memset(dgl_slots[:], 0.0)

            # ---- phase 1: projections h1T = P1 @ x.T, h2T = P2 @ x.T ------------
            with tc.tile_pool(name="pj", bufs=1, space="PSUM") as pj, \
                 tc.tile_pool(name="s1p", bufs=3) as stream:
                for q in range(4):           # token quarters of 1024
                    pa = pj.tile([128, 1024], F32, tag="pa")
                    pb = pj.tile([128, 1024], F32, tag="pb")
                    pc = pj.tile([64, 1024], F32, tag="pc")
                    for k in range(KD):
                        xq = stream.tile([128, 1024], F32R, tag="xq")
                        nc.sync.dma_start(
                            out=xq[:],
                            in_=xT.ap().bitcast(F32R)[k * 128:(k + 1) * 128,
                                        q * 1024:(q + 1) * 1024])
                        st = dict(start=(k == 0), stop=(k == KD - 1))
                        for h in range(2):
                            sl = slice(h * 512, (h + 1) * 512)
                            nc.tensor.matmul(
                                pa[:, sl],
                                lhsT=p1T_sb[:, k * R1:k * R1 + 128],
                                rhs=xq[:, sl], **st)
                            nc.tensor.matmul(
                                pb[:, sl],
                                lhsT=p1T_sb[:, k * R1 + 128:(k + 1) * R1],
                                rhs=xq[:, sl], **st)
                            nc.tensor.matmul(
                                pc[:, sl],
                                lhsT=p2T_sb[:, k * R2:(k + 1) * R2],
                                rhs=xq[:, sl], **st)
                    qs = slice(q * 1024, (q + 1) * 1024)
                    nc.vector.tensor_copy(h1T_sb[0][:, qs], pa[:])
                    nc.vector.tensor_copy(h1T_sb[1][:, qs], pb[:])
                    nc.vector.tensor_copy(h2T_sb[:, qs], pc[:])

            if phases == 1:
                # debug: reduce h1T/h2T to a scalar-ish output and stop
                dbg = res.tile([128, 1], F32)
                nc.vector.reduce_sum(dbg[:], h1T_sb[0][:].bitcast(F32),
                                     axis=mybir.AxisListType.X)
                out_sb1 = res.tile([1, 1], F32, name="dbg_out", uniquify=True)
                nc.gpsimd.tensor_reduce(out_sb1[:], dbg[:],
                                        axis=mybir.AxisListType.C,
                                        op=ALU.add)
                nc.sync.dma_start(out=out_ext.ap(), in_=out_sb1[:])

            if phases >= 2:
                # ---- phase 2: head + tail logits, exp, per-token sum-exp ------------
                NVC_H = VHC // 512   # 5 chunks of 512
                NVC_1 = V1C // 512   # 5
                NVC_2 = V2C // 512   # 3
                with tc.tile_pool(name="p2", bufs=1, space="PSUM") as p2p, \
                     tc.tile_pool(name="s2p", bufs=3) as stream:
                    for nt in range(NT):
                        xnt = stream.tile([128, KD * 128], F32R, tag="xnt")
                        nc.sync.dma_start(
                            out=xnt[:].rearrange("p (k c) -> p k c", k=KD),
                            in_=xT.ap().bitcast(F32R)
                                .rearrange("(k p) n -> p k n", p=128)
                                [:, :, nt * 128:(nt + 1) * 128])

                        ph = p2p.tile([128, VHC], F32, tag="big")
                        for k in range(KD):
                            lhs = xnt[:, k * 128:(k + 1) * 128]
                            st = dict(start=(k == 0), stop=(k == KD - 1))
                            for vc in range(NVC_H):
                                nc.tensor.matmul(
                                    ph[:, vc * 512:(vc + 1) * 512], lhsT=lhs,
                                    rhs=whT_sb[:, k * VHC + vc * 512:
                                               k * VHC + (vc + 1) * 512],
                                    **st)
                        # tail2 (3 banks) runs while exp(head) drains
                        pt2 = p2p.tile([128, V2C], F32, tag="pt2")
                        for vc in range(NVC_2):
                            nc.tensor.matmul(
                                pt2[:, vc * 512:(vc + 1) * 512],
                                lhsT=h2T_sb[:, nt * 128:(nt + 1) * 128],
                                rhs=w2T_sb[:, vc * 512:(vc + 1) * 512],
                                start=True, stop=True)
                        nc.scalar.activation(ph[:], ph[:], AF.Exp,
                                             accum_out=sh_slots[:, nt:nt + 1])
                        pt1 = p2p.tile([128, V1C], F32, tag="big")
                        for k in range(2):
                            lhs = h1T_sb[k][:, nt * 128:(nt + 1) * 128]
                            st = dict(start=(k == 0), stop=(k == 1))
                            for vc in range(NVC_1):
                                nc.tensor.matmul(
                                    pt1[:, vc * 512:(vc + 1) * 512], lhsT=lhs,
                                    rhs=w1T_sb[:, k * V1C + vc * 512:
                                               k * V1C + (vc + 1) * 512],
                                    **st)
                        nc.scalar.activation(pt2[:], pt2[:], AF.Exp,
                                             accum_out=s2_slots[:, nt:nt + 1])
                        nc.scalar.activation(pt1[:], pt1[:], AF.Exp,
                                             accum_out=s1_slots[:, nt:nt + 1])

            if phases == 2:
                dbg = res.tile([128, 1], F32)
                nc.vector.reduce_sum(dbg[:], sh_slots[:],
                                     axis=mybir.AxisListType.X)
                dbg2 = res.tile([128, 1], F32)
                nc.vector.reduce_sum(dbg2[:], s1_slots[:],
                                     axis=mybir.AxisListType.X)
                nc.vector.tensor_add(dbg[:], dbg[:], dbg2[:])
                nc.vector.reduce_sum(dbg2[:], s2_slots[:],
                                     axis=mybir.AxisListType.X)
                nc.vector.tensor_add(dbg[:], dbg[:], dbg2[:])
                out_sb1 = res.tile([1, 1], F32, name="dbg_out", uniquify=True)
                nc.gpsimd.tensor_reduce(out_sb1[:], dbg[:],
                                        axis=mybir.AxisListType.C,
                                        op=ALU.add)
                nc.sync.dma_start(out=out_ext.ap(), in_=out_sb1[:])

            if phases >= 3:
                # ---- phase 3: gathered-logit dot products (DVE) ---------------------
                dotp = tc.tile_pool(name="s3p", bufs=2)
                stream = dotp.__enter__()
                for k in range(KD):
                    xc = stream.tile([128, NSH], F32, tag="xc")
                    nc.sync.dma_start(out=xc[:],
                                      in_=xTc.ap()[k * 128:(k + 1) * 128, :])
                    wc = stream.tile([128, NSH], F32, tag="wc")
                    nc.sync.dma_start(out=wc[:],
                                      in_=wselT.ap()[k * 128:(k + 1) * 128, :])
                    scr = stream.tile([128, NSH], F32, tag="dscr", bufs=2)
                    nc.vector.tensor_mul(scr[:], xc[:], wc[:])
                    nc.vector.reduce_sum(dsh_slots[:, k:k + 1], scr[:],
                                         axis=mybir.AxisListType.X)
                CH = 1024
                for k in range(2):
                    for h in range(4):
                        w1c = stream.tile([128, CH], F32, tag="w1c")
                        nc.sync.dma_start(
                            out=w1c[:],
                            in_=w1selT.ap()[k * 128:(k + 1) * 128,
                                            h * CH:(h + 1) * CH])
                        scr2 = stream.tile([128, CH], F32, tag="dscr2", bufs=2)
                        nc.vector.tensor_mul(
                            scr2[:], h1T_sb[k][:, h * CH:(h + 1) * CH].bitcast(F32),
                            w1c[:])
                        nc.vector.reduce_sum(
                            dgl_slots[:, 4 * k + h:4 * k + h + 1], scr2[:],
                            axis=mybir.AxisListType.X)
                t2_slots = res.tile([64, 4], F32)
                nc.vector.memset(t2_slots[:], 0.0)
                for h in range(4):
                    w2c = stream.tile([64, CH], F32, tag="w2c")
                    nc.sync.dma_start(
                        out=w2c[:],
                        in_=w2selT.ap()[:, h * CH:(h + 1) * CH])
                    scr3 = stream.tile([64, CH], F32, tag="dscr2", bufs=2)
                    nc.vector.tensor_mul(
                        scr3[:], h2T_sb[:, h * CH:(h + 1) * CH].bitcast(F32),
                        w2c[:])
                    nc.vector.reduce_sum(t2_slots[:, h:h + 1], scr3[:],
                                         axis=mybir.AxisListType.X)

                dotp.__exit__(None, None, None)
                dsh_red = res.tile([128, 1], F32)
                nc.vector.reduce_sum(dsh_red[:], dsh_slots[:],
                                     axis=mybir.AxisListType.X)

            if phases == 3:
                dbg = res.tile([128, 1], F32)
                nc.vector.reduce_sum(dbg[:], dsh_slots[:],
                                     axis=mybir.AxisListType.X)
                dbg2 = res.tile([128, 1], F32)
                nc.vector.reduce_sum(dbg2[:], dgl_slots[:],
                                     axis=mybir.AxisListType.X)
                nc.vector.tensor_add(dbg[:], dbg[:], dbg2[:])
                out_sb1 = res.tile([1, 1], F32, name="dbg_out", uniquify=True)
                nc.gpsimd.tensor_reduce(out_sb1[:], dbg[:],
                                        axis=mybir.AxisListType.C,
                                        op=ALU.add)
                nc.sync.dma_start(out=out_ext.ap(), in_=out_sb1[:])

            if phases >= 4:
                # ---- phase 4: AllReduce of sum-exp partials + sharded head dot ------
                PAY = 3 * NT + 1
                pay_sb = res.tile([128, PAY], F32)
                nc.vector.tensor_copy(pay_sb[:, 0:NT], sh_slots[:])
                nc.vector.tensor_copy(pay_sb[:, NT:2 * NT], s1_slots[:])
                nc.vector.tensor_copy(pay_sb[:, 2 * NT:3 * NT], s2_slots[:])
                nc.vector.tensor_copy(pay_sb[:, 3 * NT:PAY], dsh_red[:])
                pay_dram = dram.tile([128, PAY], F32)
                red_dram = dram.tile([128, PAY], F32)
                nc.sync.dma_start(out=pay_dram[:], in_=pay_sb[:])
                nc.gpsimd.collective_compute(
                    "AllReduce", ALU.add,
                    replica_groups=[list(range(NCORES))],
                    ins=[pay_dram.opt()], outs=[red_dram.opt()])
                red_sb = res.tile([128, PAY], F32)
                nc.sync.dma_start(out=red_sb[:], in_=red_dram[:])

                # ---- phase 5: finish scalar NLL (identical on every core) -----------
                sadj = res.tile([128, 3 * NT], F32)
                nc.vector.tensor_scalar_add(sadj[:, 0:NT], red_sb[:, 0:NT],
                                            float(-PAD_H))
                nc.vector.tensor_scalar_add(sadj[:, NT:2 * NT],
                                            red_sb[:, NT:2 * NT], float(-PAD_1))
                nc.vector.tensor_scalar_add(sadj[:, 2 * NT:3 * NT],
                                            red_sb[:, 2 * NT:3 * NT], float(-PAD_2))
                logs = res.tile([128, 3 * NT], F32)
                nc.scalar.activation(logs[:], sadj[:], AF.Ln)
                lse = res.tile([128, NT], F32)
                nc.vector.tensor_mul(lse[:], logs[:, NT:2 * NT], m1_sb[:])
                t2m = res.tile([128, NT], F32)
                nc.vector.tensor_mul(t2m[:], logs[:, 2 * NT:3 * NT], m2_sb[:])
                nc.vector.tensor_add(lse[:], lse[:], logs[:, 0:NT])
                nc.vector.tensor_add(lse[:], lse[:], t2m[:])
                tot = res.tile([128, 1], F32)
                nc.vector.reduce_sum(tot[:], lse[:], axis=mybir.AxisListType.X)
                nc.vector.tensor_sub(tot[:], tot[:], red_sb[:, 3 * NT:PAY])
                dgr = res.tile([128, 1], F32)
                nc.vector.reduce_sum(dgr[:], dgl_slots[:],
                                     axis=mybir.AxisListType.X)
                nc.vector.tensor_sub(tot[:], tot[:], dgr[:])
                t2r = res.tile([64, 1], F32)
                nc.vector.reduce_sum(t2r[:], t2_slots[:],
                                     axis=mybir.AxisListType.X)
                nc.vector.tensor_sub(tot[:64, :], tot[:64, :], t2r[:])
                out_sb = res.tile([1, 1], F32)
                nc.gpsimd.tensor_reduce(out_sb[:], tot[:],
                                        axis=mybir.AxisListType.C, op=ALU.add)
                nc.sync.dma_start(out=out_ext.ap(), in_=out_sb[:])

    nc.compile()
    return nc


_NC = None


def _get_nc():
    global _NC
    if _NC is None:
        _NC = _build(phases=int(os.environ.get("KERNEL_PHASES", "4")))
    return _NC


def kernel(**inputs):
    x = np.ascontiguousarray(inputs["x"], dtype=np.float32)
    target = np.asarray(inputs["target"]).astype(np.int64)
    W_head = np.asarray(inputs["W_head"], dtype=np.float32)
    W_cluster = np.asarray(inputs["W_cluster"], dtype=np.float32)
    P1 = np.asarray(inputs["P1"], dtype=np.float32)
    W1 = np.asarray(inputs["W1"], dtype=np.float32)
    P2 = np.asarray(inputs["P2"], dtype=np.float32)
    W2 = np.asarray(inputs["W2"], dtype=np.float32)

    # ---- host-side sharding / index gathers (no arithmetic on values) ------
    W_ext = np.concatenate([W_head, W_cluster], axis=0)          # [20002, D]
    mask1 = (target >= C0) & (target < C1)
    mask2 = target >= C1
    cidx = np.where(target < C0, target,
                    np.where(mask1, C0, C0 + 1)).astype(np.int64)
    # gather from zero-padded matrices so out-of-cluster rows contribute 0
    W1p = np.concatenate([W1, np.zeros((1, R1), np.float32)], axis=0)
    W2p = np.concatenate([W2, np.zeros((1, R2), np.float32)], axis=0)
    j1 = np.where(mask1, target - C0, C1 - C0).astype(np.int64)
    j2 = np.where(mask2, target - C1, C2 - C1).astype(np.int64)

    xT = np.ascontiguousarray(x.T)                               # [D, N]
    WhT_full = np.zeros((NCORES * VHC, D), np.float32)
    WhT_full[:VH] = W_ext
    W1_full = np.zeros((NCORES * V1C, R1), np.float32)
    W1_full[:C1 - C0] = W1
    W2_full = np.zeros((NCORES * V2C, R2), np.float32)
    W2_full[:C2 - C1] = W2
    p1T = np.ascontiguousarray(P1.T)
    p2T = np.ascontiguousarray(P2.T)
    wselT = np.ascontiguousarray(W_ext[cidx].T)                  # [D, N]
    w1selT = np.ascontiguousarray(W1p[j1].T)                     # [R1, N]
    w2selT = np.ascontiguousarray(W2p[j2].T)                     # [R2, N]
    m1 = np.ascontiguousarray(
        mask1.astype(np.float32).reshape(NT, 128).T)             # [128, NT]
    m2 = np.ascontiguousarray(
        mask2.astype(np.float32).reshape(NT, 128).T)

    in_maps = []
    for i in range(NCORES):
        in_maps.append({
            "xT": xT,
            "whT": np.ascontiguousarray(
                WhT_full[i * VHC:(i + 1) * VHC].T),
            "w1T": np.ascontiguousarray(
                W1_full[i * V1C:(i + 1) * V1C].T),
            "w2T": np.ascontiguousarray(
                W2_full[i * V2C:(i + 1) * V2C].T),
            "p1T": p1T,
            "p2T": p2T,
            "xTc": np.ascontiguousarray(xT[:, i * NSH:(i + 1) * NSH]),
            "wselT": np.ascontiguousarray(wselT[:, i * NSH:(i + 1) * NSH]),
            "w1selT": w1selT,
            "w2selT": w2selT,
            "m1": m1,
            "m2": m2,
        })

    nc = _get_nc()
    trace = bool(int(os.environ.get("KERNEL_TRACE", "0")))
    if trace:
        _install_ntff_hook()
    res = run_bass_kernel_spmd(nc, in_maps, core_ids=list(range(NCORES)),
                               trace=trace)
    global LAST_EXEC_NS
    LAST_EXEC_NS = res.exec_time_ns
    val = np.float32(res.results[0]["out"][0, 0])
    return np.asarray(val, dtype=np.float32)


def _install_ntff_hook():
    """Shim antenv.axon_hooks so trace=True can capture NTFF profiles."""
    import types
    import antenv
    if hasattr(antenv, "axon_hooks"):
        return
    hooks = types.ModuleType("antenv.axon_hooks")
    holder = [None]
    hooks.set_axon_ntff_profile_hook = lambda h: holder.__setitem__(0, h)
    hooks.get_axon_ntff_profile_hook = lambda: holder[0]
    sys.modules["antenv.axon_hooks"] = hooks
    antenv.axon_hooks = hooks
    try:
        from trn_agent_boot.trn_boot import _ntff_profile_via_ctypes
        hooks.set_axon_ntff_profile_hook(
            _ntff_profile_via_ctypes("/opt/axon/libaxon_pjrt.so"))
    except Exception:
        pass



# revision 23
# speedup vs baseline: 1.7058x; 1.7058x over previous
"""Adaptive-softmax NLL loss on 8 TRN2 NeuronCores.

Strategy: tensor-parallel over the vocab dimension, all matmuls in bf16.
Each core holds its vocab slice of W_head / W1 / W2 resident in SBUF plus
the full token activations, computes exp-sums of its logit slice for all
tokens, and (token-sharded) the gathered target-logit dot products. Host
permutes tokens so cluster-1 tokens occupy the first T1 tiles and
cluster-2 tokens the next T2 tiles -- tail matmuls/exps run only on those
tiles. Two small AllReduces (one mid-kernel, one tiny at the end) combine
per-token sum-exp partials; every core then finishes the scalar NLL.

NLL = sum_n log(S_head_n) + sum_{c1} log(S_t1_n) + sum_{c2} log(S_t2_n)
      - sum_n x_n . W_ext[cidx_n] - sum_{c1} h1_n . W1[t_n-C0]
      - sum_{c2} h2_n . W2[t_n-C1]

No max-subtraction needed: logits are O(1) by construction.
"""

import os
import sys

for _p in ("/opt/trn_rl_repo",):
    if _p not in sys.path:
        sys.path.insert(0, _p)

import numpy as np
import ml_dtypes

import concourse.bacc as bacc
import concourse.bass as bass
import concourse.mybir as mybir
import concourse.tile as tile
from concourse.bass_utils import run_bass_kernel_spmd

dt = mybir.dt
AF = mybir.ActivationFunctionType
ALU = mybir.AluOpType

NCORES = 8
N, D = 4096, 1024
C0, C1, C2 = 20000, 40000, 50257
V1, V2 = C1 - C0, C2 - C1
VH = C0 + 2          # head logits incl 2 cluster columns
R1, R2 = 256, 64
VHC = 2560           # head vocab rows per core (8*2560 = 20480, pad 478)
V1C = 2560           # tail1 rows per core   (8*2560 = 20480, pad 480)
V2C = 1536           # tail2 rows per core   (8*1536 = 12288, pad 2031)
PAD_H = NCORES * VHC - VH
PAD_1 = NCORES * V1C - V1
PAD_2 = NCORES * V2C - V2
NT = N // 128        # 32 token tiles
NSH = N // NCORES    # 512 tokens per core for the sharded head dot
KD = D // 128        # 8 k-tiles over the D contraction
T1_DEF = 15          # tail1 token-tile capacity (1920 tokens, E[N1]=1630, sd 31)
T2_DEF = 9           # tail2 token-tile capacity (1152 tokens, E[N2]=836, sd 28)
NT_A = 24            # head tiles covered by the first (overlapped) AllReduce

F32, BF16 = dt.float32, dt.bfloat16
BF = ml_dtypes.bfloat16

LAST_EXEC_NS = None


def _build(T1, T2, OFF2):
    NTOK1, NTOK2 = T1 * 128, T2 * 128
    # first AllReduce fires at tile NTA-1 and must cover all tail slots
    NTA = NT_A if (T1 <= NT_A and OFF2 + T2 <= NT_A) else NT
    NTB = NT - NTA
    NOCC = bool(int(os.environ.get("KERNEL_NOCC", "0")))
    nc = bacc.Bacc("TRN2", target_bir_lowering=False, debug=False,
                   num_devices=NCORES)

    x_in = nc.declare_dram_parameter("x", [D, N], BF16, isOutput=False)
    whT = nc.declare_dram_parameter("whT", [D, VHC], BF16, isOutput=False)
    w1T = nc.declare_dram_parameter("w1T", [R1, V1C], BF16, isOutput=False)
    w2T = nc.declare_dram_parameter("w2T", [R2, V2C], BF16, isOutput=False)
    p1T = nc.declare_dram_parameter("p1T", [D, R1], BF16, isOutput=False)
    p2T = nc.declare_dram_parameter("p2T", [D, R2], BF16, isOutput=False)
    xTc = nc.declare_dram_parameter("xTc", [D, NSH], BF16, isOutput=False)
    wselT = nc.declare_dram_parameter("wselT", [D, NSH], BF16, isOutput=False)
    w1selT = nc.declare_dram_parameter("w1selT", [R1, NTOK1], BF16,
                                       isOutput=False)
    w2selT = nc.declare_dram_parameter("w2selT", [R2, NTOK2], BF16,
                                       isOutput=False)
    m1_in = nc.declare_dram_parameter("m1", [128, T1], F32, isOutput=False)
    m2_in = nc.declare_dram_parameter("m2", [128, T2], F32, isOutput=False)
    out_ext = nc.declare_dram_parameter("out", [1, 1], F32, isOutput=True)

    NHALF = N // 2

    with tile.TileContext(nc) as tc:
        with (
            tc.tile_pool(name="res", bufs=1) as res,
            tc.tile_pool(name="dram", bufs=1, space="DRAM") as dram,
        ):
            # ---- resident loads (ordered: small / phase-1 needs first) ------
            m1_sb = res.tile([128, T1], F32)
            nc.sync.dma_start(out=m1_sb[:], in_=m1_in.ap())
            m2_sb = res.tile([128, T2], F32)
            nc.sync.dma_start(out=m2_sb[:], in_=m2_in.ap())
            p1T_sb = res.tile([128, KD * R1], BF16)
            nc.sync.dma_start(
                out=p1T_sb[:].rearrange("p (k r) -> p k r", k=KD),
                in_=p1T.ap().rearrange("(k p) r -> p k r", p=128))
            p2T_sb = res.tile([128, KD * R2], BF16)
            nc.sync.dma_start(
                out=p2T_sb[:].rearrange("p (k r) -> p k r", k=KD),
                in_=p2T.ap().rearrange("(k p) r -> p k r", p=128))
            # x: two token-halves so phase 1 can start after the first
            x_lo = res.tile([128, KD * NHALF], BF16)
            nc.sync.dma_start(
                out=x_lo[:].rearrange("p (k n) -> p k n", k=KD),
                in_=x_in.ap().rearrange("(k p) n -> p k n", p=128)
                    [:, :, 0:NHALF])
            x_hi = res.tile([128, KD * NHALF], BF16)
            nc.sync.dma_start(
                out=x_hi[:].rearrange("p (k n) -> p k n", k=KD),
                in_=x_in.ap().rearrange("(k p) n -> p k n", p=128)
                    [:, :, NHALF:N])

            def xap(k, n0, sz):
                """lhsT/rhs slice of resident x: [128, sz] at D-tile k, token n0."""
                t = x_lo if n0 < NHALF else x_hi
                base = n0 if n0 < NHALF else n0 - NHALF
                return t[:, k * NHALF + base: k * NHALF + base + sz]

            # gathered-dot inputs (DVE work, overlapped with phase 2)
            xTc_sb = res.tile([128, KD * NSH], BF16)
            nc.sync.dma_start(
                out=xTc_sb[:].rearrange("p (k n) -> p k n", k=KD),
                in_=xTc.ap().rearrange("(k p) n -> p k n", p=128))
            wsel_sb = res.tile([128, KD * NSH], BF16)
            nc.sync.dma_start(
                out=wsel_sb[:].rearrange("p (k n) -> p k n", k=KD),
                in_=wselT.ap().rearrange("(k p) n -> p k n", p=128))
            w1sel_sb = res.tile([128, 2 * NTOK1], BF16)
            nc.sync.dma_start(
                out=w1sel_sb[:].rearrange("p (k n) -> p k n", k=2),
                in_=w1selT.ap().rearrange("(k p) n -> p k n", p=128))
            w2sel_sb = res.tile([64, NTOK2], BF16)
            nc.sync.dma_start(out=w2sel_sb[:], in_=w2selT.ap())

            # big weights last -- needed only once phase 2 starts
            whT_sb = res.tile([128, KD * VHC], BF16)
            nc.sync.dma_start(
                out=whT_sb[:].rearrange("p (k v) -> p k v", k=KD),
                in_=whT.ap().rearrange("(k p) v -> p k v", p=128))
            w1T_sb = res.tile([128, 2 * V1C], BF16)
            nc.sync.dma_start(
                out=w1T_sb[:].rearrange("p (k v) -> p k v", k=2),
                in_=w1T.ap().rearrange("(k p) v -> p k v", p=128))
            w2T_sb = res.tile([64, V2C], BF16)
            nc.sync.dma_start(out=w2T_sb[:], in_=w2T.ap())

            h1T_sb = [res.tile([128, N], BF16, tag=f"h1T{r}", name=f"h1T{r}")
                      for r in range(2)]
            h2T_sb = res.tile([64, N], BF16)

            # per-(tile, chunk) exp-sum slots
            shA = res.tile([128, NTA * 5], F32)        # head tiles 0..NTA-1
            shB = res.tile([128, max(NTB, 1) * 5], F32)  # head tiles NTA..
            s15 = res.tile([128, T1 * 5], F32)
            s23 = res.tile([128, T2 * 3], F32)
            dsh_slots = res.tile([128, KD], F32)   # sharded head dot partials
            dt1_slots = res.tile([128, 4], F32)    # tail1 dot partials (local)
            dt2_slots = res.tile([64, 2], F32)     # tail2 dot partials (local)

            # ---- phase 1: projections h1T = P1 @ x.T, h2T = P2 @ x.T --------
            with tc.tile_pool(name="pj", bufs=2, space="PSUM") as pj:
                for q in range(8):           # 512-token chunks
                    n0 = q * 512
                    pa = pj.tile([128, 512], F32, tag="pa")
                    pb = pj.tile([128, 512], F32, tag="pb")
                    pc = pj.tile([64, 512], F32, tag="pc")
                    for k in range(KD):
                        st = dict(start=(k == 0), stop=(k == KD - 1))
                        rhs = xap(k, n0, 512)
                        nc.tensor.matmul(
                            pa[:], lhsT=p1T_sb[:, k * R1:k * R1 + 128],
                            rhs=rhs, **st)
                        nc.tensor.matmul(
                            pb[:], lhsT=p1T_sb[:, k * R1 + 128:(k + 1) * R1],
                            rhs=rhs, **st)
                        nc.tensor.matmul(
                            pc[:], lhsT=p2T_sb[:, k * R2:(k + 1) * R2],
                            rhs=rhs, **st)
                    qs = slice(n0, n0 + 512)
                    nc.vector.tensor_copy(h1T_sb[0][:, qs], pa[:])
                    nc.vector.tensor_copy(h1T_sb[1][:, qs], pb[:])
                    nc.vector.tensor_copy(h2T_sb[:, qs], pc[:])

            # ---- gathered-logit dots on DVE (runs during phase 2) -----------
            scr = res.tile([128, 512], F32)
            scr1 = res.tile([128, NTOK1 // 2], F32)
            scr2 = res.tile([64, NTOK2 // 2], F32)
            for k in range(KD):
                nc.vector.tensor_mul(
                    scr[:], xTc_sb[:, k * NSH:(k + 1) * NSH],
                    wsel_sb[:, k * NSH:(k + 1) * NSH])
                nc.vector.reduce_sum(dsh_slots[:, k:k + 1], scr[:],
                                     axis=mybir.AxisListType.X)
            for k in range(2):
                for h in range(2):
                    hs = slice(h * (NTOK1 // 2), (h + 1) * (NTOK1 // 2))
                    nc.vector.tensor_mul(
                        scr1[:], h1T_sb[k][:, hs],
                        w1sel_sb[:, k * NTOK1 + h * (NTOK1 // 2):
                                 k * NTOK1 + (h + 1) * (NTOK1 // 2)])
                    nc.vector.reduce_sum(
                        dt1_slots[:, 2 * k + h:2 * k + h + 1], scr1[:],
                        axis=mybir.AxisListType.X)
            for h in range(2):
                hw = NTOK2 // 2
                nc.vector.tensor_mul(
                    scr2[:],
                    h2T_sb[:, OFF2 * 128 + h * hw:OFF2 * 128 + (h + 1) * hw],
                    w2sel_sb[:, h * hw:(h + 1) * hw])
                nc.vector.reduce_sum(dt2_slots[:, h:h + 1], scr2[:],
                                     axis=mybir.AxisListType.X)

            # ---- phase 2: head + tail logits, exp, per-token sum-exp --------
            pay_a = res.tile([128, NTA + T1 + T2 + 1], F32)
            red_a = res.tile([128, NTA + T1 + T2 + 1], F32)
            paydA = dram.tile([128, NTA + T1 + T2 + 1], F32)
            reddA = dram.tile([128, NTA + T1 + T2 + 1], F32)
            nllA = res.tile([128, 1], F32)
            logsA = res.tile([128, NTA + T1 + T2], F32)
            if NTB:
                pay_b = res.tile([128, NTB], F32)
                red_b = res.tile([128, NTB], F32)
                paydB = dram.tile([128, NTB], F32)
                reddB = dram.tile([128, NTB], F32)
                logsB = res.tile([128, NTB], F32)

            with tc.tile_pool(name="p2", bufs=2, space="PSUM") as p2p:
                for nt in range(NT):
                    n0 = nt * 128
                    sh = shA if nt < NTA else shB
                    sc = (nt if nt < NTA else nt - NTA) * 5
                    for vc in range(5):
                        hb = p2p.tile([128, 512], F32, tag="hb")
                        for k in range(KD):
                            nc.tensor.matmul(
                                hb[:], lhsT=xap(k, n0, 128),
                                rhs=whT_sb[:, k * VHC + vc * 512:
                                           k * VHC + (vc + 1) * 512],
                                start=(k == 0), stop=(k == KD - 1))
                        nc.scalar.activation(hb[:], hb[:], AF.Exp,
                                             accum_out=sh[:, sc + vc:sc + vc + 1])
                    if nt < T1:
                        for vc in range(5):
                            tb = p2p.tile([128, 512], F32, tag="tb")
                            for k in range(2):
                                nc.tensor.matmul(
                                    tb[:],
                                    lhsT=h1T_sb[k][:, n0:n0 + 128],
                                    rhs=w1T_sb[:, k * V1C + vc * 512:
                                               k * V1C + (vc + 1) * 512],
                                    start=(k == 0), stop=(k == 1))
                            nc.scalar.activation(
                                tb[:], tb[:], AF.Exp,
                                accum_out=s15[:, nt * 5 + vc:nt * 5 + vc + 1])
                    if OFF2 <= nt < OFF2 + T2:
                        for vc in range(3):
                            ub = p2p.tile([128, 512], F32, tag="ub")
                            nc.tensor.matmul(
                                ub[:], lhsT=h2T_sb[:, n0:n0 + 128],
                                rhs=w2T_sb[:, vc * 512:(vc + 1) * 512],
                                start=True, stop=True)
                            nc.scalar.activation(
                                ub[:], ub[:], AF.Exp,
                                accum_out=s23[:, (nt - OFF2) * 3 + vc:
                                              (nt - OFF2) * 3 + vc + 1])

                    if nt == NTA - 1:
                        # ---- first AllReduce: head tiles 0..NTA-1 + tails ---
                        shA5 = shA[:].rearrange("p (t v) -> p t v", v=5)
                        nc.vector.tensor_add(pay_a[:, 0:NTA], shA5[:, :, 0],
                                             shA5[:, :, 1])
                        for v in range(2, 5):
                            nc.vector.tensor_add(pay_a[:, 0:NTA],
                                                 pay_a[:, 0:NTA],
                                                 shA5[:, :, v])
                        s155 = s15[:].rearrange("p (t v) -> p t v", v=5)
                        nc.vector.tensor_add(pay_a[:, NTA:NTA + T1],
                                             s155[:, :, 0], s155[:, :, 1])
                        for v in range(2, 5):
                            nc.vector.tensor_add(pay_a[:, NTA:NTA + T1],
                                                 pay_a[:, NTA:NTA + T1],
                                                 s155[:, :, v])
                        s233 = s23[:].rearrange("p (t v) -> p t v", v=3)
                        nc.vector.tensor_add(pay_a[:, NTA + T1:NTA + T1 + T2],
                                             s233[:, :, 0], s233[:, :, 1])
                        nc.vector.tensor_add(pay_a[:, NTA + T1:NTA + T1 + T2],
                                             pay_a[:, NTA + T1:NTA + T1 + T2],
                                             s233[:, :, 2])
                        nc.vector.reduce_sum(
                            pay_a[:, NTA + T1 + T2:NTA + T1 + T2 + 1],
                            dsh_slots[:], axis=mybir.AxisListType.X)
                        nc.sync.dma_start(out=paydA[:], in_=pay_a[:])
                        if NOCC:
                            nc.sync.dma_start(out=reddA[:], in_=paydA[:])
                        else:
                            nc.gpsimd.collective_compute(
                                "AllReduce", ALU.add,
                                replica_groups=[list(range(NCORES))],
                                ins=[paydA.opt()], outs=[reddA.opt()])
                        nc.sync.dma_start(out=red_a[:], in_=reddA[:])

            # ---- post-loop: log/mask math for AR-A (overlapped AR done) -----
            nc.vector.tensor_scalar_add(
                logsA[:, 0:NTA], red_a[:, 0:NTA], float(-PAD_H))
            nc.vector.tensor_scalar_add(
                logsA[:, NTA:NTA + T1],
                red_a[:, NTA:NTA + T1], float(-PAD_1))
            nc.vector.tensor_scalar_add(
                logsA[:, NTA + T1:NTA + T1 + T2],
                red_a[:, NTA + T1:NTA + T1 + T2], float(-PAD_2))
            nc.scalar.activation(logsA[:], logsA[:], AF.Ln)
            nc.vector.tensor_mul(logsA[:, NTA:NTA + T1],
                                 logsA[:, NTA:NTA + T1], m1_sb[:])
            nc.vector.tensor_mul(
                logsA[:, NTA + T1:NTA + T1 + T2],
                logsA[:, NTA + T1:NTA + T1 + T2], m2_sb[:])
            nc.vector.reduce_sum(nllA[:], logsA[:],
                                 axis=mybir.AxisListType.X)

            # ---- final: second AllReduce (remaining head tiles) + NLL -------
            tot = res.tile([128, 1], F32)
            if NTB:
                shB5 = shB[:].rearrange("p (t v) -> p t v", v=5)
                nc.vector.tensor_add(pay_b[:], shB5[:, :, 0], shB5[:, :, 1])
                for v in range(2, 5):
                    nc.vector.tensor_add(pay_b[:], pay_b[:], shB5[:, :, v])
                nc.sync.dma_start(out=paydB[:], in_=pay_b[:])
                if NOCC:
                    nc.sync.dma_start(out=reddB[:], in_=paydB[:])
                else:
                    nc.gpsimd.collective_compute(
                        "AllReduce", ALU.add,
                        replica_groups=[list(range(NCORES))],
                        ins=[paydB.opt()], outs=[reddB.opt()])
                nc.sync.dma_start(out=red_b[:], in_=reddB[:])
                nc.vector.tensor_scalar_add(logsB[:], red_b[:], float(-PAD_H))
                nc.scalar.activation(logsB[:], logsB[:], AF.Ln)
                nc.vector.reduce_sum(tot[:], logsB[:],
                                     axis=mybir.AxisListType.X)
                nc.vector.tensor_add(tot[:], tot[:], nllA[:])
            else:
                nc.vector.tensor_copy(tot[:], nllA[:])
            nc.vector.tensor_sub(tot[:], tot[:],
                                 red_a[:, NTA + T1 + T2:NTA + T1 + T2 + 1])
            dgr = res.tile([128, 1], F32)
            nc.vector.reduce_sum(dgr[:], dt1_slots[:],
                                 axis=mybir.AxisListType.X)
            nc.vector.tensor_sub(tot[:], tot[:], dgr[:])
            t2r = res.tile([64, 1], F32)
            nc.vector.reduce_sum(t2r[:], dt2_slots[:],
                                 axis=mybir.AxisListType.X)
            nc.vector.tensor_sub(tot[:64, :], tot[:64, :], t2r[:])
            out_sb = res.tile([1, 1], F32)
            nc.gpsimd.tensor_reduce(out_sb[:], tot[:],
                                    axis=mybir.AxisListType.C, op=ALU.add)
            nc.sync.dma_start(out=out_ext.ap(), in_=out_sb[:])

    nc.compile()
    return nc


_NC = {}


def _get_nc(T1, T2, OFF2):
    key = (T1, T2, OFF2)
    if key not in _NC:
        _NC[key] = _build(T1, T2, OFF2)
    return _NC[key]


def _prepare(inputs):
    x = np.ascontiguousarray(inputs["x"], dtype=np.float32)
    target = np.asarray(inputs["target"]).astype(np.int64)
    W_head = np.asarray(inputs["W_head"], dtype=np.float32)
    W_cluster = np.asarray(inputs["W_cluster"], dtype=np.float32)
    P1 = np.asarray(inputs["P1"], dtype=np.float32)
    W1 = np.asarray(inputs["W1"], dtype=np.float32)
    P2 = np.asarray(inputs["P2"], dtype=np.float32)
    W2 = np.asarray(inputs["W2"], dtype=np.float32)

    # ---- host-side sharding / permutation / index gathers ------------------
    mask1 = (target >= C0) & (target < C1)
    mask2 = target >= C1
    mask0 = ~(mask1 | mask2)
    idx1 = np.nonzero(mask1)[0]
    idx2 = np.nonzero(mask2)[0]
    idx0 = np.nonzero(mask0)[0]
    N1, N2 = len(idx1), len(idx2)
    T1 = max(T1_DEF, -(-N1 // 128))
    T2 = max(T2_DEF, -(-N2 // 128))
    if T1 * 128 + T2 * 128 <= N:
        OFF2 = T1
        NTOK1, NTOK2 = T1 * 128, T2 * 128
        # layout: [cluster1 | pad0 | cluster2 | rest of cluster0]
        perm = np.empty(N, dtype=np.int64)
        g1 = NTOK1 - N1                   # cluster-0 fill between 1 and 2
        perm[0:N1] = idx1
        perm[N1:NTOK1] = idx0[:g1]
        perm[NTOK1:NTOK1 + N2] = idx2
        perm[NTOK1 + N2:] = idx0[g1:]
    else:
        # degenerate fallback: no permutation, tails run over all tiles
        T1 = T2 = NT
        OFF2 = 0
        NTOK1, NTOK2 = N, N
        perm = np.arange(N, dtype=np.int64)

    xp = x[perm]
    tp = target[perm]
    m1p = (tp[:NTOK1] >= C0) & (tp[:NTOK1] < C1)
    m2p = tp[OFF2 * 128:OFF2 * 128 + NTOK2] >= C1
    cidx = np.where(tp < C0, tp,
                    np.where(tp < C1, C0, C0 + 1)).astype(np.int64)
    W_ext = np.concatenate([W_head, W_cluster], axis=0)          # [20002, D]
    W1p = np.concatenate([W1, np.zeros((1, R1), np.float32)], axis=0)
    W2p = np.concatenate([W2, np.zeros((1, R2), np.float32)], axis=0)
    j1 = np.where(m1p, tp[:NTOK1] - C0, V1).astype(np.int64)
    j2 = np.where(m2p, tp[OFF2 * 128:OFF2 * 128 + NTOK2] - C1,
                  V2).astype(np.int64)

    xT = np.ascontiguousarray(xp.T.astype(BF))                   # [D, N]
    WhT_full = np.zeros((NCORES * VHC, D), np.float32)
    WhT_full[:VH] = W_ext
    W1_full = np.zeros((NCORES * V1C, R1), np.float32)
    W1_full[:V1] = W1
    W2_full = np.zeros((NCORES * V2C, R2), np.float32)
    W2_full[:V2] = W2
    wselT = np.ascontiguousarray(W_ext[cidx].T.astype(BF))       # [D, N]
    w1selT = np.ascontiguousarray(W1p[j1].T.astype(BF))          # [R1, NTOK1]
    w2selT = np.ascontiguousarray(W2p[j2].T.astype(BF))          # [R2, NTOK2]
    m1 = np.ascontiguousarray(
        m1p.astype(np.float32).reshape(T1, 128).T)               # [128, T1]
    m2 = np.ascontiguousarray(
        m2p.astype(np.float32).reshape(T2, 128).T)

    in_maps = []
    for i in range(NCORES):
        in_maps.append({
            "x": xT,
            "whT": np.ascontiguousarray(
                WhT_full[i * VHC:(i + 1) * VHC].T.astype(BF)),
            "w1T": np.ascontiguousarray(
                W1_full[i * V1C:(i + 1) * V1C].T.astype(BF)),
            "w2T": np.ascontiguousarray(
                W2_full[i * V2C:(i + 1) * V2C].T.astype(BF)),
            "p1T": np.ascontiguousarray(P1.T.astype(BF)),
            "p2T": np.ascontiguousarray(P2.T.astype(BF)),
            "xTc": np.ascontiguousarray(xT[:, i * NSH:(i + 1) * NSH]),
            "wselT": np.ascontiguousarray(wselT[:, i * NSH:(i + 1) * NSH]),
            "w1selT": w1selT,
            "w2selT": w2selT,
            "m1": m1,
            "m2": m2,
        })
    return in_maps, T1, T2, OFF2


def kernel(**inputs):
    in_maps, T1, T2, OFF2 = _prepare(inputs)
    nc = _get_nc(T1, T2, OFF2)
    trace = bool(int(os.environ.get("KERNEL_TRACE", "0")))
    if trace:
        _install_ntff_hook()
    res = run_bass_kernel_spmd(nc, in_maps, core_ids=list(range(NCORES)),
                               trace=trace)
    global LAST_EXEC_NS
    LAST_EXEC_NS = res.exec_time_ns
    val = np.float32(res.results[0]["out"][0, 0])
    return np.asarray(val, dtype=np.float32)


def _install_ntff_hook():
    """Shim antenv.axon_hooks so trace=True can capture NTFF profiles."""
    import types
    import antenv
    if hasattr(antenv, "axon_hooks"):
        return
    hooks = types.ModuleType("antenv.axon_hooks")
    holder = [None]
    hooks.set_axon_ntff_profile_hook = lambda h: holder.__setitem__(0, h)
    hooks.get_axon_ntff_profile_hook = lambda: holder[0]
    sys.modules["antenv.axon_hooks"] = hooks
    antenv.axon_hooks = hooks
    try:
        from trn_agent_boot.trn_boot import _ntff_profile_via_ctypes
        hooks.set_axon_ntff_profile_hook(
            _ntff_profile_via_ctypes("/opt/axon/libaxon_pjrt.so"))
    except Exception:
        pass


# revision 24
# speedup vs baseline: 2.9725x; 1.7426x over previous
"""Adaptive-softmax NLL loss on 8 TRN2 NeuronCores.

Strategy: tensor-parallel over the vocab dimension. Head / tail1 GEMMs run
in fp8e4m3 DoubleRow mode (K=256 per pass, 0.5 cycles/row), tail2 in bf16.
Weights are pre-scaled by 32 on the host so fp8 stays in the normal range;
the exp activation compensates with scale=1/32. Each core holds its vocab
slice of W_head / W1 / W2 and all token activations resident in SBUF,
computes exp-sums of its logit slice (exp on the scalar engine into bf16
scratch, per-chunk sums on DVE), plus token-sharded gathered target-logit
dot products on DVE. Host permutes tokens so cluster-1 tokens occupy the
first T1 tiles and cluster-2 tokens the next T2 tiles -- tail work runs
only on those tiles. Two AllReduces (one hidden mid-kernel, one tiny at
the end) combine per-token sum-exp partials; every core then finishes the
scalar NLL identically.

NLL = sum_n log(S_head_n) + sum_{c1} log(S_t1_n) + sum_{c2} log(S_t2_n)
      - sum_n x_n . W_ext[cidx_n] - sum_{c1} h1_n . W1[t_n-C0]
      - sum_{c2} h2_n . W2[t_n-C1]

No max-subtraction needed: logits are O(1) by construction.
"""

import os
import sys

for _p in ("/opt/trn_rl_repo",):
    if _p not in sys.path:
        sys.path.insert(0, _p)

import numpy as np
import ml_dtypes

import concourse.bacc as bacc
import concourse.bass as bass
import concourse.mybir as mybir
import concourse.tile as tile
from concourse.bass_utils import run_bass_kernel_spmd

dt = mybir.dt
AF = mybir.ActivationFunctionType
ALU = mybir.AluOpType
DR = mybir.MatmulPerfMode.DoubleRow

NCORES = 8
N, D = 4096, 1024
C0, C1, C2 = 20000, 40000, 50257
V1, V2 = C1 - C0, C2 - C1
VH = C0 + 2          # head logits incl 2 cluster columns
R1, R2 = 256, 64
VHC = 2560           # head vocab rows per core (8*2560 = 20480, pad 478)
V1C = 2560           # tail1 rows per core   (8*2560 = 20480, pad 480)
V2C = 1536           # tail2 rows per core   (8*1536 = 12288, pad 2031)
PAD_H = NCORES * VHC - VH
PAD_1 = NCORES * V1C - V1
PAD_2 = NCORES * V2C - V2
NT = N // 128        # 32 token tiles
NSH = N // NCORES    # 512 tokens per core for the sharded head dot
KD = D // 128        # 8 k-tiles over the D contraction
KK = KD // 2         # 4 DoubleRow passes over D
T1_DEF = 15          # tail1 token-tile capacity (1920 tokens, E[N1]=1630)
T2_DEF = 9           # tail2 token-tile capacity (1152 tokens, E[N2]=836)
NT_A = 24            # head tiles covered by the first (overlapped) AllReduce
WSC = 32.0           # fp8 weight pre-scale; exp() compensates by 1/WSC

F32, BF16, FP8 = dt.float32, dt.bfloat16, dt.float8e4
BF = ml_dtypes.bfloat16
F8 = ml_dtypes.float8_e4m3fn

# head / tail1 vocab chunking: (offset, width, psum tag)
CH_H = [(0, 1024, "A"), (1024, 1024, "A"), (2048, 512, "B")]
CH_2 = [(0, 1024, "A"), (1024, 512, "B")]
NCH_H = len(CH_H)    # slots per head / tail1 tile
NCH_2 = len(CH_2)    # slots per tail2 tile

LAST_EXEC_NS = None


def _build(T1, T2, OFF2):
    NTOK1, NTOK2 = T1 * 128, T2 * 128
    # first AllReduce fires at tile NTA-1 and must cover all tail slots
    NTA = NT_A if (T1 <= NT_A and OFF2 + T2 <= NT_A) else NT
    NTB = NT - NTA
    NOCC = bool(int(os.environ.get("KERNEL_NOCC", "0")))
    nc = bacc.Bacc("TRN2", target_bir_lowering=False, debug=False,
                   num_devices=NCORES)

    x_in = nc.declare_dram_parameter("x", [D, N], FP8, isOutput=False)
    whT = nc.declare_dram_parameter("whT", [D, VHC], FP8, isOutput=False)
    w1T = nc.declare_dram_parameter("w1T", [R1, V1C], FP8, isOutput=False)
    w2T = nc.declare_dram_parameter("w2T", [R2, V2C], BF16, isOutput=False)
    p1T = nc.declare_dram_parameter("p1T", [D, R1], FP8, isOutput=False)
    p2T = nc.declare_dram_parameter("p2T", [D, R2], FP8, isOutput=False)
    xTc = nc.declare_dram_parameter("xTc", [D, NSH], BF16, isOutput=False)
    wselT = nc.declare_dram_parameter("wselT", [D, NSH], BF16, isOutput=False)
    w1selT = nc.declare_dram_parameter("w1selT", [R1, NTOK1], BF16,
                                       isOutput=False)
    w2selT = nc.declare_dram_parameter("w2selT", [R2, NTOK2], BF16,
                                       isOutput=False)
    m1_in = nc.declare_dram_parameter("m1", [128, T1], F32, isOutput=False)
    m2_in = nc.declare_dram_parameter("m2", [128, T2], F32, isOutput=False)
    out_ext = nc.declare_dram_parameter("out", [1, 1], F32, isOutput=True)

    with tile.TileContext(nc) as tc:
        with (
            tc.tile_pool(name="res", bufs=1) as res,
            tc.tile_pool(name="dram", bufs=1, space="DRAM") as dram,
        ):
            # ---- resident loads (ordered: phase-1 needs first) --------------
            m1_sb = res.tile([128, T1], F32)
            nc.sync.dma_start(out=m1_sb[:], in_=m1_in.ap())
            m2_sb = res.tile([128, T2], F32)
            nc.sync.dma_start(out=m2_sb[:], in_=m2_in.ap())
            p1_sb = res.tile([128, KD * R1], FP8)
            nc.sync.dma_start(
                out=p1_sb[:].rearrange("p (k r) -> p k r", k=KD),
                in_=p1T.ap().rearrange("(k p) r -> p k r", p=128))
            p2_sb = res.tile([128, KD * R2], FP8)
            nc.sync.dma_start(
                out=p2_sb[:].rearrange("p (k r) -> p k r", k=KD),
                in_=p2T.ap().rearrange("(k p) r -> p k r", p=128))
            # x in fp8, 8 token-slices of 512 so phase 1 starts early
            x_sl = []
            for q in range(8):
                xs_ = res.tile([128, KD * 512], FP8, tag=f"x{q}",
                               name=f"x{q}")
                nc.sync.dma_start(
                    out=xs_[:].rearrange("p (k n) -> p k n", k=KD),
                    in_=x_in.ap().rearrange("(k p) n -> p k n", p=128)
                        [:, :, q * 512:(q + 1) * 512])
                x_sl.append(xs_)

            def xdr(kk, n0, sz):
                """x DoubleRow AP [p, 2, sz] at DR pass kk, token n0."""
                q, off = divmod(n0, 512)
                return (x_sl[q][:].rearrange("p (k n) -> p k n", k=KD)
                        [:, 2 * kk:2 * kk + 2, off:off + sz])

            # gathered-dot inputs (DVE work, overlapped with phase 2)
            xTc_sb = res.tile([128, KD * NSH], BF16)
            nc.sync.dma_start(
                out=xTc_sb[:].rearrange("p (k n) -> p k n", k=KD),
                in_=xTc.ap().rearrange("(k p) n -> p k n", p=128))
            wsel_sb = res.tile([128, KD * NSH], BF16)
            nc.sync.dma_start(
                out=wsel_sb[:].rearrange("p (k n) -> p k n", k=KD),
                in_=wselT.ap().rearrange("(k p) n -> p k n", p=128))
            w1sel_sb = res.tile([128, 2 * NTOK1], BF16)
            nc.sync.dma_start(
                out=w1sel_sb[:].rearrange("p (k n) -> p k n", k=2),
                in_=w1selT.ap().rearrange("(k p) n -> p k n", p=128))
            w2sel_sb = res.tile([64, NTOK2], BF16)
            nc.sync.dma_start(out=w2sel_sb[:], in_=w2selT.ap())

            # big weights -- needed once phase 2 starts
            wh_sb = res.tile([128, KD * VHC], FP8)
            nc.sync.dma_start(
                out=wh_sb[:].rearrange("p (k v) -> p k v", k=KD),
                in_=whT.ap().rearrange("(k p) v -> p k v", p=128))
            wh4 = wh_sb[:].rearrange("p (k i v) -> p k i v", k=KK, i=2)
            w1_sb = res.tile([128, 2 * V1C], FP8)
            nc.sync.dma_start(
                out=w1_sb[:].rearrange("p (i v) -> p i v", i=2),
                in_=w1T.ap().rearrange("(i p) v -> p i v", p=128))
            w12 = w1_sb[:].rearrange("p (i v) -> p i v", i=2)
            w2_sb = res.tile([64, V2C], BF16)
            nc.sync.dma_start(out=w2_sb[:], in_=w2T.ap())

            h1T_sb = [res.tile([128, N], BF16, tag=f"h1T{r}", name=f"h1T{r}")
                      for r in range(2)]
            h1f8 = res.tile([128, 2 * N], FP8)
            h18 = h1f8[:].rearrange("p (i n) -> p i n", i=2)
            h2T_sb = res.tile([64, N], BF16)

            # per-(tile, chunk) exp-sum slots
            shA = res.tile([128, NTA * NCH_H], F32)
            shB = res.tile([128, max(NTB, 1) * NCH_H], F32)
            s15 = res.tile([128, T1 * NCH_H], F32)
            s23 = res.tile([128, T2 * NCH_2], F32)
            dsh_slots = res.tile([128, KD], F32)   # sharded head dot partials
            dt1_slots = res.tile([128, 4], F32)    # tail1 dot partials (local)
            dt2_slots = res.tile([64, 2], F32)     # tail2 dot partials (local)

            # ---- phase 1: projections h1 = P1 @ x.T, h2 = P2 @ x.T (fp8) ----
            p1r = p1_sb[:].rearrange("p (k r) -> p k r", k=KD)
            p2r = p2_sb[:].rearrange("p (k r) -> p k r", k=KD)
            with tc.tile_pool(name="pj", bufs=2, space="PSUM") as pj:
                for q in range(8):           # 512-token chunks
                    n0 = q * 512
                    pa = pj.tile([128, 512], F32, tag="pa")
                    pb = pj.tile([128, 512], F32, tag="pb")
                    pc = pj.tile([64, 512], F32, tag="pc")
                    for kk in range(KK):
                        st = dict(start=(kk == 0), stop=(kk == KK - 1),
                                  perf_mode=DR)
                        rhs = xdr(kk, n0, 512)
                        nc.tensor.matmul(
                            pa[:], lhsT=p1r[:, 2 * kk:2 * kk + 2, 0:128],
                            rhs=rhs, **st)
                        nc.tensor.matmul(
                            pb[:], lhsT=p1r[:, 2 * kk:2 * kk + 2, 128:256],
                            rhs=rhs, **st)
                        nc.tensor.matmul(
                            pc[:], lhsT=p2r[:, 2 * kk:2 * kk + 2, 0:64],
                            rhs=rhs, **st)
                    qs = slice(n0, n0 + 512)
                    nc.vector.tensor_scalar_mul(h1T_sb[0][:, qs], pa[:],
                                                1.0 / WSC)
                    nc.vector.tensor_scalar_mul(h1T_sb[1][:, qs], pb[:],
                                                1.0 / WSC)
                    nc.vector.tensor_scalar_mul(h18[:, 0, qs], pa[:],
                                                1.0 / WSC)
                    nc.vector.tensor_scalar_mul(h18[:, 1, qs], pb[:],
                                                1.0 / WSC)
                    nc.vector.tensor_scalar_mul(h2T_sb[:, qs], pc[:],
                                                1.0 / WSC)

            # ---- gathered-logit dots on DVE (run during phase 2) ------------
            scr = res.tile([128, 512], F32)
            scr1 = res.tile([128, NTOK1 // 2], F32)
            scr2 = res.tile([64, NTOK2 // 2], F32)
            for k in range(KD):
                nc.vector.tensor_mul(
                    scr[:], xTc_sb[:, k * NSH:(k + 1) * NSH],
                    wsel_sb[:, k * NSH:(k + 1) * NSH])
                nc.vector.reduce_sum(dsh_slots[:, k:k + 1], scr[:],
                                     axis=mybir.AxisListType.X)
            for k in range(2):
                for h in range(2):
                    hs = slice(h * (NTOK1 // 2), (h + 1) * (NTOK1 // 2))
                    nc.vector.tensor_mul(
                        scr1[:], h1T_sb[k][:, hs],
                        w1sel_sb[:, k * NTOK1 + h * (NTOK1 // 2):
                                 k * NTOK1 + (h + 1) * (NTOK1 // 2)])
                    nc.vector.reduce_sum(
                        dt1_slots[:, 2 * k + h:2 * k + h + 1], scr1[:],
                        axis=mybir.AxisListType.X)
            for h in range(2):
                hw = NTOK2 // 2
                nc.vector.tensor_mul(
                    scr2[:],
                    h2T_sb[:, OFF2 * 128 + h * hw:OFF2 * 128 + (h + 1) * hw],
                    w2sel_sb[:, h * hw:(h + 1) * hw])
                nc.vector.reduce_sum(dt2_slots[:, h:h + 1], scr2[:],
                                     axis=mybir.AxisListType.X)

            # ---- phase 2: head + tail logits, exp, per-token sum-exp --------
            pay_a = res.tile([128, NTA + T1 + T2 + 1], F32)
            red_a = res.tile([128, NTA + T1 + T2 + 1], F32)
            paydA = dram.tile([128, NTA + T1 + T2 + 1], F32)
            reddA = dram.tile([128, NTA + T1 + T2 + 1], F32)
            nllA = res.tile([128, 1], F32)
            logsA = res.tile([128, NTA + T1 + T2], F32)
            if NTB:
                pay_b = res.tile([128, NTB], F32)
                red_b = res.tile([128, NTB], F32)
                paydB = dram.tile([128, NTB], F32)
                reddB = dram.tile([128, NTB], F32)
                logsB = res.tile([128, NTB], F32)

            with tc.tile_pool(name="p2", bufs=2, space="PSUM") as p2p, \
                 tc.tile_pool(name="xsp", bufs=4) as xsp:
                for nt in range(NT):
                    n0 = nt * 128
                    sh = shA if nt < NTA else shB
                    sc = (nt if nt < NTA else nt - NTA) * NCH_H
                    for ci, (c0, cw, tg) in enumerate(CH_H):
                        pA = p2p.tile([128, cw], F32, tag=tg)
                        for kk in range(KK):
                            for h in range(cw // 512):
                                nc.tensor.matmul(
                                    pA[:, h * 512:(h + 1) * 512],
                                    lhsT=xdr(kk, n0, 128),
                                    rhs=wh4[:, kk, :,
                                            c0 + h * 512:c0 + (h + 1) * 512],
                                    start=(kk == 0), stop=(kk == KK - 1),
                                    perf_mode=DR)
                        xs = xsp.tile([128, cw], BF16,
                                      tag=f"xs{cw}", bufs=2)
                        nc.scalar.activation(xs[:], pA[:], AF.Exp,
                                             scale=1.0 / WSC)
                        nc.vector.reduce_sum(sh[:, sc + ci:sc + ci + 1],
                                             xs[:], axis=mybir.AxisListType.X)
                    if nt < T1:
                        for ci, (c0, cw, tg) in enumerate(CH_H):
                            pA = p2p.tile([128, cw], F32, tag=tg)
                            for h in range(cw // 512):
                                nc.tensor.matmul(
                                    pA[:, h * 512:(h + 1) * 512],
                                    lhsT=h18[:, :, n0:n0 + 128],
                                    rhs=w12[:, :,
                                            c0 + h * 512:c0 + (h + 1) * 512],
                                    start=True, stop=True, perf_mode=DR)
                            xs = xsp.tile([128, cw], BF16,
                                          tag=f"xs{cw}", bufs=2)
                            nc.scalar.activation(xs[:], pA[:], AF.Exp,
                                                 scale=1.0 / WSC)
                            nc.vector.reduce_sum(
                                s15[:, nt * NCH_H + ci:nt * NCH_H + ci + 1],
                                xs[:], axis=mybir.AxisListType.X)
                    if OFF2 <= nt < OFF2 + T2:
                        for ci, (c0, cw, tg) in enumerate(CH_2):
                            pA = p2p.tile([128, cw], F32, tag=tg)
                            for h in range(cw // 512):
                                nc.tensor.matmul(
                                    pA[:, h * 512:(h + 1) * 512],
                                    lhsT=h2T_sb[:, n0:n0 + 128],
                                    rhs=w2_sb[:,
                                              c0 + h * 512:c0 + (h + 1) * 512],
                                    start=True, stop=True)
                            xs = xsp.tile([128, cw], BF16,
                                          tag=f"xs{cw}", bufs=2)
                            nc.scalar.activation(xs[:], pA[:], AF.Exp)
                            nc.vector.reduce_sum(
                                s23[:, (nt - OFF2) * NCH_2 + ci:
                                    (nt - OFF2) * NCH_2 + ci + 1],
                                xs[:], axis=mybir.AxisListType.X)

                    if nt == NTA - 1:
                        # ---- first AllReduce: head tiles 0..NTA-1 + tails ---
                        shA5 = shA[:].rearrange("p (t v) -> p t v", v=NCH_H)
                        nc.vector.tensor_add(pay_a[:, 0:NTA], shA5[:, :, 0],
                                             shA5[:, :, 1])
                        for v in range(2, NCH_H):
                            nc.vector.tensor_add(pay_a[:, 0:NTA],
                                                 pay_a[:, 0:NTA],
                                                 shA5[:, :, v])
                        s155 = s15[:].rearrange("p (t v) -> p t v", v=NCH_H)
                        nc.vector.tensor_add(pay_a[:, NTA:NTA + T1],
                                             s155[:, :, 0], s155[:, :, 1])
                        for v in range(2, NCH_H):
                            nc.vector.tensor_add(pay_a[:, NTA:NTA + T1],
                                                 pay_a[:, NTA:NTA + T1],
                                                 s155[:, :, v])
                        s233 = s23[:].rearrange("p (t v) -> p t v", v=NCH_2)
                        nc.vector.tensor_add(pay_a[:, NTA + T1:NTA + T1 + T2],
                                             s233[:, :, 0], s233[:, :, 1])
                        for v in range(2, NCH_2):
                            nc.vector.tensor_add(
                                pay_a[:, NTA + T1:NTA + T1 + T2],
                                pay_a[:, NTA + T1:NTA + T1 + T2],
                                s233[:, :, v])
                        nc.vector.reduce_sum(
                            pay_a[:, NTA + T1 + T2:NTA + T1 + T2 + 1],
                            dsh_slots[:], axis=mybir.AxisListType.X)
                        nc.sync.dma_start(out=paydA[:], in_=pay_a[:])
                        if NOCC:
                            nc.sync.dma_start(out=reddA[:], in_=paydA[:])
                        else:
                            nc.gpsimd.collective_compute(
                                "AllReduce", ALU.add,
                                replica_groups=[list(range(NCORES))],
                                ins=[paydA.opt()], outs=[reddA.opt()])
                        nc.sync.dma_start(out=red_a[:], in_=reddA[:])

            # ---- post-loop: log/mask math for AR-A (overlapped AR done) -----
            nc.vector.tensor_scalar_add(
                logsA[:, 0:NTA], red_a[:, 0:NTA], float(-PAD_H))
            nc.vector.tensor_scalar_add(
                logsA[:, NTA:NTA + T1],
                red_a[:, NTA:NTA + T1], float(-PAD_1))
            nc.vector.tensor_scalar_add(
                logsA[:, NTA + T1:NTA + T1 + T2],
                red_a[:, NTA + T1:NTA + T1 + T2], float(-PAD_2))
            nc.scalar.activation(logsA[:], logsA[:], AF.Ln)
            nc.vector.tensor_mul(logsA[:, NTA:NTA + T1],
                                 logsA[:, NTA:NTA + T1], m1_sb[:])
            nc.vector.tensor_mul(
                logsA[:, NTA + T1:NTA + T1 + T2],
                logsA[:, NTA + T1:NTA + T1 + T2], m2_sb[:])
            nc.vector.reduce_sum(nllA[:], logsA[:],
                                 axis=mybir.AxisListType.X)

            # ---- final: second AllReduce (remaining head tiles) + NLL -------
            tot = res.tile([128, 1], F32)
            if NTB:
                shB5 = shB[:].rearrange("p (t v) -> p t v", v=NCH_H)
                nc.vector.tensor_add(pay_b[:], shB5[:, :, 0], shB5[:, :, 1])
                for v in range(2, NCH_H):
                    nc.vector.tensor_add(pay_b[:], pay_b[:], shB5[:, :, v])
                nc.sync.dma_start(out=paydB[:], in_=pay_b[:])
                if NOCC:
                    nc.sync.dma_start(out=reddB[:], in_=paydB[:])
                else:
                    nc.gpsimd.collective_compute(
                        "AllReduce", ALU.add,
                        replica_groups=[list(range(NCORES))],
                        ins=[paydB.opt()], outs=[reddB.opt()])
                nc.sync.dma_start(out=red_b[:], in_=reddB[:])
                nc.vector.tensor_scalar_add(logsB[:], red_b[:], float(-PAD_H))
                nc.scalar.activation(logsB[:], logsB[:], AF.Ln)
                nc.vector.reduce_sum(tot[:], logsB[:],
                                     axis=mybir.AxisListType.X)
                nc.vector.tensor_add(tot[:], tot[:], nllA[:])
            else:
                nc.vector.tensor_copy(tot[:], nllA[:])
            nc.vector.tensor_sub(tot[:], tot[:],
                                 red_a[:, NTA + T1 + T2:NTA + T1 + T2 + 1])
            dgr = res.tile([128, 1], F32)
            nc.vector.reduce_sum(dgr[:], dt1_slots[:],
                                 axis=mybir.AxisListType.X)
            nc.vector.tensor_sub(tot[:], tot[:], dgr[:])
            t2r = res.tile([64, 1], F32)
            nc.vector.reduce_sum(t2r[:], dt2_slots[:],
                                 axis=mybir.AxisListType.X)
            nc.vector.tensor_sub(tot[:64, :], tot[:64, :], t2r[:])
            out_sb = res.tile([1, 1], F32)
            nc.gpsimd.tensor_reduce(out_sb[:], tot[:],
                                    axis=mybir.AxisListType.C, op=ALU.add)
            nc.sync.dma_start(out=out_ext.ap(), in_=out_sb[:])

    nc.compile()
    return nc


_NC = {}


def _get_nc(T1, T2, OFF2):
    key = (T1, T2, OFF2)
    if key not in _NC:
        _NC[key] = _build(T1, T2, OFF2)
    return _NC[key]


def _prepare(inputs):
    x = np.ascontiguousarray(inputs["x"], dtype=np.float32)
    target = np.asarray(inputs["target"]).astype(np.int64)
    W_head = np.asarray(inputs["W_head"], dtype=np.float32)
    W_cluster = np.asarray(inputs["W_cluster"], dtype=np.float32)
    P1 = np.asarray(inputs["P1"], dtype=np.float32)
    W1 = np.asarray(inputs["W1"], dtype=np.float32)
    P2 = np.asarray(inputs["P2"], dtype=np.float32)
    W2 = np.asarray(inputs["W2"], dtype=np.float32)

    # ---- host-side sharding / permutation / index gathers ------------------
    mask1 = (target >= C0) & (target < C1)
    mask2 = target >= C1
    mask0 = ~(mask1 | mask2)
    idx1 = np.nonzero(mask1)[0]
    idx2 = np.nonzero(mask2)[0]
    idx0 = np.nonzero(mask0)[0]
    N1, N2 = len(idx1), len(idx2)
    T1 = max(T1_DEF, -(-N1 // 128))
    T2 = max(T2_DEF, -(-N2 // 128))
    if T1 * 128 + T2 * 128 <= N:
        OFF2 = T1
        NTOK1, NTOK2 = T1 * 128, T2 * 128
        # layout: [cluster1 | pad0 | cluster2 | rest of cluster0]
        perm = np.empty(N, dtype=np.int64)
        g1 = NTOK1 - N1                   # cluster-0 fill between 1 and 2
        perm[0:N1] = idx1
        perm[N1:NTOK1] = idx0[:g1]
        perm[NTOK1:NTOK1 + N2] = idx2
        perm[NTOK1 + N2:] = idx0[g1:]
    else:
        # degenerate fallback: no permutation, tails run over all tiles
        T1 = T2 = NT
        OFF2 = 0
        NTOK1, NTOK2 = N, N
        perm = np.arange(N, dtype=np.int64)

    xp = x[perm]
    tp = target[perm]
    m1p = (tp[:NTOK1] >= C0) & (tp[:NTOK1] < C1)
    m2p = tp[OFF2 * 128:OFF2 * 128 + NTOK2] >= C1
    cidx = np.where(tp < C0, tp,
                    np.where(tp < C1, C0, C0 + 1)).astype(np.int64)
    W_ext = np.concatenate([W_head, W_cluster], axis=0)          # [20002, D]
    W1p = np.concatenate([W1, np.zeros((1, R1), np.float32)], axis=0)
    W2p = np.concatenate([W2, np.zeros((1, R2), np.float32)], axis=0)
    j1 = np.where(m1p, tp[:NTOK1] - C0, V1).astype(np.int64)
    j2 = np.where(m2p, tp[OFF2 * 128:OFF2 * 128 + NTOK2] - C1,
                  V2).astype(np.int64)

    xT = np.ascontiguousarray(xp.T)                              # [D, N] f32
    xT8 = np.ascontiguousarray(xT.astype(F8))
    xTb = np.ascontiguousarray(xT.astype(BF))
    WhT_full = np.zeros((NCORES * VHC, D), np.float32)
    WhT_full[:VH] = W_ext
    W1_full = np.zeros((NCORES * V1C, R1), np.float32)
    W1_full[:V1] = W1
    W2_full = np.zeros((NCORES * V2C, R2), np.float32)
    W2_full[:V2] = W2
    wselT = np.ascontiguousarray(W_ext[cidx].T.astype(BF))       # [D, N]
    w1selT = np.ascontiguousarray(W1p[j1].T.astype(BF))          # [R1, NTOK1]
    w2selT = np.ascontiguousarray(W2p[j2].T.astype(BF))          # [R2, NTOK2]
    m1 = np.ascontiguousarray(
        m1p.astype(np.float32).reshape(T1, 128).T)               # [128, T1]
    m2 = np.ascontiguousarray(
        m2p.astype(np.float32).reshape(T2, 128).T)

    in_maps = []
    for i in range(NCORES):
        in_maps.append({
            "x": xT8,
            "whT": np.ascontiguousarray(
                (WhT_full[i * VHC:(i + 1) * VHC].T * WSC).astype(F8)),
            "w1T": np.ascontiguousarray(
                (W1_full[i * V1C:(i + 1) * V1C].T * WSC).astype(F8)),
            "w2T": np.ascontiguousarray(
                W2_full[i * V2C:(i + 1) * V2C].T.astype(BF)),
            "p1T": np.ascontiguousarray((P1.T * WSC).astype(F8)),
            "p2T": np.ascontiguousarray((P2.T * WSC).astype(F8)),
            "xTc": np.ascontiguousarray(xTb[:, i * NSH:(i + 1) * NSH]),
            "wselT": np.ascontiguousarray(wselT[:, i * NSH:(i + 1) * NSH]),
            "w1selT": w1selT,
            "w2selT": w2selT,
            "m1": m1,
            "m2": m2,
        })
    return in_maps, T1, T2, OFF2


def kernel(**inputs):
    in_maps, T1, T2, OFF2 = _prepare(inputs)
    nc = _get_nc(T1, T2, OFF2)
    trace = bool(int(os.environ.get("KERNEL_TRACE", "0")))
    if trace:
        _install_ntff_hook()
    res = run_bass_kernel_spmd(nc, in_maps, core_ids=list(range(NCORES)),
                               trace=trace)
    global LAST_EXEC_NS
    LAST_EXEC_NS = res.exec_time_ns
    val = np.float32(res.results[0]["out"][0, 0])
    return np.asarray(val, dtype=np.float32)


def _install_ntff_hook():
    """Shim antenv.axon_hooks so trace=True can capture NTFF profiles."""
    import types
    import antenv
    if hasattr(antenv, "axon_hooks"):
        return
    hooks = types.ModuleType("antenv.axon_hooks")
    holder = [None]
    hooks.set_axon_ntff_profile_hook = lambda h: holder.__setitem__(0, h)
    hooks.get_axon_ntff_profile_hook = lambda: holder[0]
    sys.modules["antenv.axon_hooks"] = hooks
    antenv.axon_hooks = hooks
    try:
        from trn_agent_boot.trn_boot import _ntff_profile_via_ctypes
        hooks.set_axon_ntff_profile_hook(
            _ntff_profile_via_ctypes("/opt/axon/libaxon_pjrt.so"))
    except Exception:
        pass
